# revision 11
# baseline (speedup 1.0000x reference)
"""DisenGCN-style 4-layer GCN on 8 Trainium2 NeuronCores (v3).

Algorithm (matches reference.py):
    src,dst,norm = gcn_norm(edge_index)  with self loops, norm=dinv[src]*dinv[dst]
    h = BN(relu(conv(x, W1)));  h = BN(relu(conv(h, W2)))
    h = BN(relu(conv(h, W3)));  out = log_softmax(conv(h, W4))

Key algebraic restructuring (v3):
  * tables carry U = dinv * r (RAW relu output, no BN affine, no GEMM).
    BN's per-feature affine (a, bb) and the next layer's weight GEMM both
    commute through the per-feature-linear aggregation:
        z_{l+1}[d] = W^T( a (.) (dinv[d] * sum_s U[s]) + dinv[d]*sigma[d]*bb ) + b
    with sigma[d] = sum_{s in N~(d)} dinv[s] STATIC (precomputed host-side).
    No table-build phase exists: tables are written per dst-block during
    message passing and the AllGather streams out chunk by chunk while the
    layer is still running.  The rank-1 bias term is one extra tiny matmul
    per block (lhsT = [1,D] W^T bb, rhs = [1,P] s-tilde row).
  * conv1 propagates x before its GEMM (linearity).

Distribution (8 cores, SPMD single program):
  * nodes block-partitioned with a quarter-striped global layout:
      gpos = q*bs + c*qr + j*128 + s      (qr = npc/4, j = block in quarter)
    so int16 gather bucket k == AllGather chunk k: chunk-q's collective
    output IS the gather table for bucket q.
  * per-layer AllGather = 4 chunk collectives, each fired right after the
    owning quarter's dst blocks finish their post step -> overlaps mp.
  * per-edge gather uses dma_gather (int16 indices, 256B rows); edges grouped
    (dst quarter, src bucket, dst block); gather calls batch up to CALL_TILES
    tiles (SWDGE descriptor scratch enlarged to match).
  * scatter-add is a one-hot matmul: per 128-edge tile, S[e, slot] =
    (dst_slot[e] == slot) built on DVE; PSUM holds a quarter's 25 live
    accumulators.
  * BN stats via a tiny AllReduce at layer end; a folds into W' = a (.) W
    (one DVE op) consumed by the NEXT layer's post step.
"""

import os
import sys
import math
import numpy as np

sys.path.insert(0, "/opt/trn_rl_repo")

P = 128
NQ = 4                 # quarters == buckets == AG chunks
GB = 6                 # dst blocks per PSUM batch
CALL_TILES = 8         # max tiles per dma_gather call (1024-idx ucode limit)
SWDGE_SCRATCH = 49152  # bytes/partition for the SWDGE descriptor ring
MP_MODE = "full"       # debug: gather | smat | mm | full
SKIP_AG = False        # debug: skip inter-layer AllGathers (wrong results)
SKIP_AR = False        # debug: skip BN AllReduces (wrong results)
SINGLE_PACKET = True   # dma_gather single_packet flag


# ---------------------------------------------------------------- host prep


def _build_plan(edge_index, N, n_cores):
    """Partition edges; build per-core index/slot streams + shared structure."""
    npc = int(math.ceil(N / n_cores / P)) * P               # nodes per core
    npad = npc * n_cores
    nb = npc // P                                           # blocks per core
    bs = npad // NQ                                         # bucket rows
    assert npad % NQ == 0 and bs <= 32768

    src0 = np.asarray(edge_index[0], dtype=np.int64)
    dst0 = np.asarray(edge_index[1], dtype=np.int64)

    # relabel nodes so each (core, block) bin carries a near-equal edge load:
    # greedy heaviest-first assignment to the lightest non-full bin.
    import heapq
    degN = np.bincount(dst0, minlength=N)
    nbins = n_cores * nb
    order = np.argsort(-degN, kind="stable")
    heap = [(0, b) for b in range(nbins)]
    heapq.heapify(heap)
    cap = np.full(nbins, P, np.int64)

    base = (np.arange(nbins) // nb) * npc + (np.arange(nbins) % nb) * P
    newid = np.empty(npad, np.int64)
    for n in order:
        while True:
            load, b = heapq.heappop(heap)
            if cap[b] > 0:
                break
        newid[n] = base[b] + (P - cap[b])
        cap[b] -= 1
        heapq.heappush(heap, (load + int(degN[n]), b))
    spots = np.concatenate([np.arange(base[b] + P - cap[b], base[b] + P)
                            for b in range(nbins)]) if cap.sum() else \
        np.empty(0, np.int64)
    newid[N:] = spots
    src = newid[src0]
    dst = newid[dst0]

    # degree includes the self loop
    deg = np.bincount(dst, minlength=npad).astype(np.float64)
    deg[newid[:N]] += 1.0
    dinv = np.zeros(npad, np.float32)
    nz = deg > 0
    dinv[nz] = (1.0 / np.sqrt(deg[nz])).astype(np.float32)

    # sigma[d] = dinv[d] + sum_{e: dst=d} dinv[src_e];  s~ = dinv * sigma
    sigma = dinv.astype(np.float64).copy()
    np.add.at(sigma, dst, dinv[src].astype(np.float64))
    stld = (dinv * sigma.astype(np.float32)).astype(np.float32)

    ngrp = nb * NQ                        # (dst block, bucket) groups
    core_data = []
    counts = np.zeros((n_cores, ngrp), np.int64)
    for c in range(n_cores):
        rem = dst - c * npc
        own = (rem >= 0) & (rem < npc)
        s = src[own]
        d_rem = rem[own]
        blk = d_rem // P                  # dst block 0..nb-1
        slot = d_rem & (P - 1)
        k = s // bs                       # src bucket
        sloc = s - k * bs                 # row within bucket table
        # stream order: (batch of GB dst blocks, bucket, dst block-in-batch)
        skey = (blk // GB) * (NQ * GB) + k * GB + (blk % GB)
        o = np.argsort(skey, kind="stable")
        gid = blk * NQ + k                # group id for counting
        counts[c] = np.bincount(gid, minlength=ngrp)
        core_data.append((skey[o], sloc[o], slot[o].astype(np.float32)))

    tiles_grp = (counts.max(axis=0) + P - 1) // P   # shared tile structure

    def skey_to_gid(sk):
        B, r = divmod(sk, NQ * GB)
        k, j = divmod(r, GB)
        return (B * GB + j) * NQ + k

    n_batches = (nb + GB - 1) // GB
    batches = []      # one per batch of GB dst blocks
    grp_start = np.full(ngrp, -1, np.int64)
    tpos = 0
    for B in range(n_batches):
        blks = list(range(B * GB, min((B + 1) * GB, nb)))
        b_t0 = tpos
        bcalls = []
        btiles = {b: [] for b in blks}
        for k in range(NQ):
            c_t0 = tpos
            for b in blks:
                gid = b * NQ + k
                t = int(tiles_grp[gid])
                if t:
                    grp_start[gid] = tpos * P
                    btiles[b].append((k, tpos, t))
                    tpos += t
            for sub in range(c_t0, tpos, CALL_TILES):
                bcalls.append((k, sub, min(CALL_TILES, tpos - sub)))
        batches.append(dict(t0=b_t0, nt=tpos - b_t0, blks=blks,
                            calls=bcalls, btiles=btiles))
    tott = tpos
    tote = tott * P

    idx_arrs, slot_arrs = [], []
    for c in range(n_cores):
        skey_s, sloc_s, slot_s = core_data[c]
        ne = len(skey_s)
        uniq, first = np.unique(skey_s, return_index=True)
        run_first = np.zeros(ne, np.int64)
        run_first[first] = first
        run_first = np.maximum.accumulate(run_first)
        within = np.arange(ne, dtype=np.int64) - run_first
        gids = np.array([skey_to_gid(int(u)) for u in uniq])
        gs = np.repeat(grp_start[gids], np.diff(np.append(first, ne)))
        posi = gs + within
        idx_stream = np.zeros(tote, np.int16)
        slot_stream = np.full(tote, -1.0, np.float32)
        idx_stream[posi] = sloc_s.astype(np.int16)
        slot_stream[posi] = slot_s
        idx_arrs.append(np.ascontiguousarray(
            np.tile(idx_stream.reshape(-1, 16).T, (8, 1))))
        slot_arrs.append(np.ascontiguousarray(slot_stream.reshape(-1, P).T))

    return dict(
        N=N, n_cores=n_cores, npc=npc, npad=npad, nb=nb,
        bs=bs, batches=batches, tott=tott, dinv=dinv, stld=stld, newid=newid,
        idx_arrs=idx_arrs, slot_arrs=slot_arrs,
    )


# ------------------------------------------------------------ bass program


class _Stop(Exception):
    pass


def _build_nc(plan, stop_after=None, repeat=1):
    from concourse import bass, mybir, tile, bacc
    f32 = mybir.dt.float32
    f16 = mybir.dt.float16
    i16 = mybir.dt.int16
    Alu = mybir.AluOpType
    Act = mybir.ActivationFunctionType
    Axis = mybir.AxisListType

    npc, npad, nb, bs = (plan[k] for k in ("npc", "npad", "nb", "bs"))
    tott = plan["tott"]
    n_cores = plan["n_cores"]
    N = plan["N"]
    rg = [list(range(n_cores))]
    batches = plan["batches"]
    max_bt = max(b["nt"] for b in batches)
    F1, F2, F3 = 128, 128, 64            # post-conv (BN) widths
    FIN = {1: 64, 2: F1, 3: F2, 4: F3}   # aggregated content width per layer
    FOUT = {1: F1, 2: F2, 3: F3, 4: 2}   # conv output width per layer

    nc = bacc.Bacc("TRN2", target_bir_lowering=False, debug=False,
                   num_devices=n_cores, num_swdge_queues=4,
                   dynamic_dma_scratch_size=SWDGE_SCRATCH)

    # ---- I/O ----
    totc = tott * 8
    x_own = nc.declare_dram_parameter("x_own", [npc, 64], f32, isOutput=False)
    idx_p = nc.declare_dram_parameter("idx", [P, totc], i16, isOutput=False)
    slot_p = nc.declare_dram_parameter("slot", [P, tott], f16, isOutput=False)
    dinv_rep_p = nc.declare_dram_parameter("dinv_rep", [P, npc], f32,
                                           isOutput=False)
    dinv_nm_p = nc.declare_dram_parameter("dinv_nm", [P, nb], f32,
                                          isOutput=False)
    srow_p = nc.declare_dram_parameter("srow", [1, npc], f16, isOutput=False)
    coliota_p = nc.declare_dram_parameter("coliota", [P, P], f16,
                                          isOutput=False)
    ident16_p = nc.declare_dram_parameter("ident16", [P, P], f16,
                                          isOutput=False)
    ident_p = nc.declare_dram_parameter("ident", [P, P], f32, isOutput=False)
    W1_p = nc.declare_dram_parameter("W1p", [64, 128], f16, isOutput=False)
    W2_p = nc.declare_dram_parameter("W2", [128, 128], f32, isOutput=False)
    W3_p = nc.declare_dram_parameter("W3", [128, 64], f32, isOutput=False)
    W4_p = nc.declare_dram_parameter("W4", [64, 2], f32, isOutput=False)
    vecs = {}
    for nm, f in (("b1", F1), ("g1", F1), ("be1", F1), ("cs1", F1), ("cq1", F1),
                  ("b2", F2), ("g2", F2), ("be2", F2), ("cs2", F2), ("cq2", F2),
                  ("b3", F3), ("g3", F3), ("be3", F3), ("cs3", F3), ("cq3", F3)):
        vecs[nm] = nc.declare_dram_parameter(nm, [f, 1], f32, isOutput=False)
    b4r_p = nc.declare_dram_parameter("b4r", [P, 2], f32, isOutput=False)
    out_p = nc.declare_dram_parameter("out", [P, nb, 2], f32, isOutput=True)

    # ---- internal DRAM ----
    T = {l: nc.dram_tensor(f"T{l}", [npad, P], f16, addr_space="Shared")
         for l in (1, 2, 3, 4)}
    Tc = {l: nc.dram_tensor(f"T{l}c", [npc, P], f16) for l in (1, 2, 3, 4)}
    st_in = {l: nc.dram_tensor(f"stin{l}", [[F1, F2, F3][l - 1], 2], f32)
             for l in (1, 2, 3)}
    st_out = {l: nc.dram_tensor(f"stout{l}", [[F1, F2, F3][l - 1], 2], f32,
                                addr_space="Shared") for l in (1, 2, 3)}

    with tile.TileContext(nc) as tc:
        with (
            tc.tile_pool(name="const", bufs=1) as cpool,
            tc.tile_pool(name="slotp", bufs=1) as slpool,
            tc.tile_pool(name="dinvrep", bufs=1) as dpool,
            tc.tile_pool(name="idxs", bufs=2) as ipool,
            tc.tile_pool(name="msg", bufs=5) as mpool,
            tc.tile_pool(name="smat", bufs=4) as spool,
            tc.tile_pool(name="selfp", bufs=4) as selfpool,
            tc.tile_pool(name="blk", bufs=4) as bpool,
            tc.tile_pool(name="stats", bufs=1) as stpool,
            tc.tile_pool(name="psa", bufs=GB, space="PSUM") as ps_acc,
            tc.tile_pool(name="psg", bufs=1, space="PSUM") as ps_gem,
            tc.tile_pool(name="pst", bufs=1, space="PSUM") as ps_tr,
        ):
            # ---------- constants ----------
            coliota = cpool.tile([P, 1, P], f16, tag="coliota")
            nc.sync.dma_start(out=coliota[:], in_=coliota_p[:, None, :])
            ident16 = cpool.tile([P, P], f16, tag="ident16")
            nc.sync.dma_start(out=ident16[:], in_=ident16_p[:])
            ident = cpool.tile([P, P], f32, tag="ident")
            nc.sync.dma_start(out=ident[:], in_=ident_p[:])
            W1s = cpool.tile([64, 128], f16, tag="W1p")
            nc.sync.dma_start(out=W1s[:], in_=W1_p[:])
            W2s = cpool.tile([128, 128], f32, tag="W2")
            nc.sync.dma_start(out=W2s[:], in_=W2_p[:])
            W3s = cpool.tile([128, 64], f32, tag="W3")
            nc.sync.dma_start(out=W3s[:], in_=W3_p[:])
            W4s = cpool.tile([64, 2], f32, tag="W4")
            nc.sync.dma_start(out=W4s[:], in_=W4_p[:])
            b4r = cpool.tile([P, 1, 2], f32, tag="b4r")
            nc.sync.dma_start(out=b4r[:], in_=b4r_p[:, None, :])
            vt = {}
            for nm, h in vecs.items():
                t = cpool.tile(list(h.shape), f32, tag=nm)
                nc.sync.dma_start(out=t[:], in_=h[:])
                vt[nm] = t
            dinv_nm = cpool.tile([P, nb], f32, tag="dinv_nm")
            nc.sync.dma_start(out=dinv_nm[:], in_=dinv_nm_p[:])
            srow = cpool.tile([1, npc], f16, tag="srow")
            nc.sync.dma_start(out=srow[:], in_=srow_p[:])

            slot_sb = slpool.tile([P, tott, 1], f16)
            nc.sync.dma_start(out=slot_sb[:], in_=slot_p[:, :, None])
            dinv_rep = dpool.tile([P, npc], f32)
            nc.sync.dma_start(out=dinv_rep[:], in_=dinv_rep_p[:])

            def ck(name):
                if stop_after == name:
                    raise _Stop()

            # zero the pad columns of the narrow tables once (layers 1, 4)
            zpad = cpool.tile([P, 64], f16, tag="zpad")
            nc.vector.memset(zpad[:], 0.0)
            for li in (1, 4):
                for b in range(nb):
                    nc.sync.dma_start(out=Tc[li][b * P:(b + 1) * P, 64:],
                                      in_=zpad[:])

            # ---------- U1 = dinv * x_pad (own slice) -> chunked AllGather
            for b in range(nb):
                xb = bpool.tile([P, 64], f32, tag="xb")
                nc.sync.dma_start(out=xb[:], in_=x_own[b * P:(b + 1) * P, :])
                t1b = bpool.tile([P, 64], f16, tag="t1b")
                nc.vector.tensor_tensor(
                    out=t1b[:], in0=xb[:],
                    in1=dinv_nm[:, b:b + 1].to_broadcast([P, 64]),
                    op=Alu.mult)
                nc.sync.dma_start(out=Tc[1][b * P:(b + 1) * P, :64],
                                  in_=t1b[:])
            nc.gpsimd.collective_compute(
                "AllGather", Alu.bypass, replica_groups=rg,
                ins=[Tc[1].ap().opt()], outs=[T[1].ap().opt()])

            Wp = {}    # folded lhsT weights per layer (from bn_phase)
            wbb = {}   # W^T bb row per layer

            gq = [0]   # global gather counter: queue follows DMASW lane mod 4

            def message_pass(l, post_block):
                tcn = Tc[l]
                for bi, binfo in enumerate(batches):
                    bt0, bnt, blks = binfo["t0"], binfo["nt"], binfo["blks"]
                    if bnt == 0:
                        continue
                    idxb = ipool.tile([P, max_bt * 8], i16, tag="idxb")
                    nc.sync.dma_start(out=idxb[:, :bnt * 8],
                                      in_=idx_p[:, bt0 * 8:(bt0 + bnt) * 8])
                    msgs = {}
                    for ci, (k, ct0, cnt) in enumerate(binfo["calls"]):
                        nidx = cnt * P
                        msgs[ci] = mpool.tile([P, CALL_TILES, P], f16,
                                              tag="msg", name=f"msg{ci}")
                        nc.gpsimd.dma_gather(
                            out_ap=msgs[ci][:, :cnt, :],
                            in_ap=T[l][k * bs:(k + 1) * bs, :],
                            idxs_ap=idxb[:, (ct0 - bt0) * 8:
                                         (ct0 - bt0 + cnt) * 8],
                            num_idxs=nidx, num_idxs_reg=nidx, elem_size=P,
                            single_packet=SINGLE_PACKET,
                            queue_num=gq[0] % 4)
                        gq[0] += 1
                    if MP_MODE == "gather":
                        continue
                    accs = {}
                    first = {b: False for b in blks}
                    last_mm = {}
                    for b in blks:
                        accs[b] = ps_acc.tile([P, P], f32, tag="acc",
                                              name=f"acc{b}")
                        bt = binfo["btiles"][b]
                        last_mm[b] = bt[-1][1] + bt[-1][2] - 1 if bt else None
                        loc0 = b * P
                        ms = selfpool.tile([P, P], f16, tag="ms",
                                           name=f"ms{b}")
                        nc.sync.dma_start(out=ms[:],
                                          in_=tcn[loc0:loc0 + P, :])
                        nc.tensor.matmul(out=accs[b][:], lhsT=ms[:],
                                         rhs=ident16[:],
                                         start=True, stop=(last_mm[b] is None))
                    for ci, (k, ct0, cnt) in enumerate(binfo["calls"]):
                        S = spool.tile([P, CALL_TILES, P], f16, tag="S",
                                       name=f"S{ci}")
                        nc.vector.tensor_tensor(
                            out=S[:, :cnt, :],
                            in0=slot_sb[:, ct0:ct0 + cnt, :]
                                .to_broadcast([P, cnt, P]),
                            in1=coliota[:].to_broadcast([P, cnt, P]),
                            op=Alu.is_equal)
                        if MP_MODE == "smat":
                            continue
                        for b in blks:
                            for (kk, t0, nt) in binfo["btiles"][b]:
                                if kk != k:
                                    continue
                                lo = max(t0, ct0)
                                hi = min(t0 + nt, ct0 + cnt)
                                for t in range(lo, hi):
                                    nc.tensor.matmul(
                                        out=accs[b][:],
                                        lhsT=msgs[ci][:, t - ct0, :],
                                        rhs=S[:, t - ct0, :],
                                        start=first[b],
                                        stop=(t == last_mm[b]))
                                    first[b] = False
                    if MP_MODE in ("smat", "mm"):
                        continue
                    for b in blks:
                        post_block(b, accs[b])
                # single AllGather of the next layer's table
                if l < 4 and not SKIP_AG:
                    nc.gpsimd.collective_compute(
                        "AllGather", Alu.bypass, replica_groups=rg,
                        ins=[Tc[l + 1].ap().opt()], outs=[T[l + 1].ap().opt()])

            def post_factory(l, ssum, ssq, logits=None):
                fin = FIN[l]
                fout = FOUT[l]

                def post(b, acc):
                    loc0 = b * P
                    t16 = bpool.tile([fin, P], f16, tag="t16")
                    nc.vector.tensor_tensor(
                        out=t16[:], in0=acc[:fin, :],
                        in1=dinv_rep[:fin, loc0:loc0 + P], op=Alu.mult)
                    if l == 4:
                        zt = ps_gem.tile([128, P], f32, tag="z")
                        nc.tensor.matmul(out=zt[:, :2], lhsT=t16[:],
                                         rhs=Wp[4][:], start=True, stop=False)
                        nc.tensor.matmul(out=zt[:, :2],
                                         lhsT=srow[:, loc0:loc0 + P],
                                         rhs=wbb[4][:, :2], start=False,
                                         stop=True)
                        nc.vector.tensor_copy(out=logits[:, b, :],
                                              in_=zt[:, :2])
                        return
                    z = ps_gem.tile([128, P], f32, tag="z")
                    if l == 1:
                        nc.tensor.matmul(out=z[:fout, :], lhsT=W1s[:],
                                         rhs=t16[:], start=True, stop=True)
                    else:
                        nc.tensor.matmul(out=z[:fout, :], lhsT=Wp[l][:],
                                         rhs=t16[:], start=True, stop=False)
                        nc.tensor.matmul(out=z[:fout, :],
                                         lhsT=wbb[l][:, :fout],
                                         rhs=srow[:, loc0:loc0 + P],
                                         start=False, stop=True)
                    r = bpool.tile([fout, P], f32, tag="r")
                    nc.scalar.activation(out=r[:], in_=z[:fout, :],
                                         func=Act.Relu,
                                         bias=vt[f"b{l}"][:], scale=1.0,
                                         accum_out=ssum[:, b:b + 1])
                    sq = bpool.tile([fout, P], f16, tag="sq")
                    nc.scalar.activation(out=sq[:], in_=r[:], func=Act.Square,
                                         accum_out=ssq[:, b:b + 1])
                    pt = ps_tr.tile([P, 128], f32, tag="pt")
                    nc.tensor.transpose(out=pt[:, :fout], in_=r[:],
                                        identity=ident[:fout, :fout])
                    ut = bpool.tile([P, fout], f16, tag="ut")
                    nc.vector.tensor_tensor(
                        out=ut[:], in0=pt[:, :fout],
                        in1=dinv_nm[:, b:b + 1].to_broadcast([P, fout]),
                        op=Alu.mult)
                    nc.sync.dma_start(out=Tc[l + 1][loc0:loc0 + P, :fout],
                                      in_=ut[:])
                return post

            def bn_phase(l, F, ssum, ssq):
                stat = stpool.tile([F, 2], f32, tag=f"stat{l}")
                nc.vector.tensor_reduce(out=stat[:, 0:1], in_=ssum[:],
                                        axis=Axis.X, op=Alu.add)
                nc.vector.tensor_reduce(out=stat[:, 1:2], in_=ssq[:],
                                        axis=Axis.X, op=Alu.add)
                nc.vector.tensor_tensor(out=stat[:, 0:1], in0=stat[:, 0:1],
                                        in1=vt[f"cs{l}"][:], op=Alu.add)
                nc.vector.tensor_tensor(out=stat[:, 1:2], in0=stat[:, 1:2],
                                        in1=vt[f"cq{l}"][:], op=Alu.add)
                nc.sync.dma_start(out=st_in[l][:], in_=stat[:])
                if not SKIP_AR:
                    nc.gpsimd.collective_compute(
                        "AllReduce", Alu.add, replica_groups=rg,
                        ins=[st_in[l].ap().opt()], outs=[st_out[l].ap().opt()])
                stg = stpool.tile([F, 2], f32, tag=f"statg{l}")
                nc.sync.dma_start(out=stg[:],
                                  in_=(st_in[l][:] if SKIP_AR else st_out[l][:]))
                mean = stpool.tile([F, 1], f32, tag=f"mean{l}")
                nc.vector.tensor_scalar(out=mean[:], in0=stg[:, 0:1],
                                        scalar1=1.0 / N, scalar2=None,
                                        op0=Alu.mult)
                var = stpool.tile([F, 1], f32, tag=f"var{l}")
                nc.vector.tensor_scalar(out=var[:], in0=stg[:, 1:2],
                                        scalar1=1.0 / N, scalar2=None,
                                        op0=Alu.mult)
                msq = stpool.tile([F, 1], f32, tag=f"msq{l}")
                nc.vector.tensor_tensor(out=msq[:], in0=mean[:], in1=mean[:],
                                        op=Alu.mult)
                nc.vector.tensor_tensor(out=var[:], in0=var[:], in1=msq[:],
                                        op=Alu.subtract)
                ve = stpool.tile([F, 1], f32, tag=f"ve{l}")
                nc.vector.tensor_scalar(out=ve[:], in0=var[:], scalar1=1e-5,
                                        scalar2=None, op0=Alu.add)
                sd = stpool.tile([F, 1], f32, tag=f"sd{l}")
                nc.scalar.activation(out=sd[:], in_=ve[:], func=Act.Sqrt)
                inv = stpool.tile([F, 1], f32, tag=f"inv{l}")
                nc.vector.reciprocal(out=inv[:], in_=sd[:])
                a = stpool.tile([F, 1], f32, tag=f"a{l}")
                nc.vector.tensor_tensor(out=a[:], in0=vt[f"g{l}"][:],
                                        in1=inv[:], op=Alu.mult)
                am = stpool.tile([F, 1], f32, tag=f"am{l}")
                nc.vector.tensor_tensor(out=am[:], in0=a[:], in1=mean[:],
                                        op=Alu.mult)
                bb = stpool.tile([F, 1], f32, tag=f"bb{l}")
                nc.vector.tensor_tensor(out=bb[:], in0=vt[f"be{l}"][:],
                                        in1=am[:], op=Alu.subtract)
                # fold into next layer: Wp = a (.) W  (f16 lhsT), wbb = bb^T W
                Wn = {1: W2s, 2: W3s, 3: W4s}[l]
                Dn = {1: F2, 2: F3, 3: 2}[l]
                wp = stpool.tile([F, Dn], f16, tag=f"wp{l}")
                nc.vector.tensor_tensor(out=wp[:], in0=Wn[:],
                                        in1=a[:].to_broadcast([F, Dn]),
                                        op=Alu.mult)
                Wp[l + 1] = wp
                wb_ps = ps_gem.tile([128, P], f32, tag="z")
                nc.tensor.matmul(out=wb_ps[0:1, :Dn], lhsT=bb[:], rhs=Wn[:],
                                 start=True, stop=True)
                wb = stpool.tile([1, 128], f16, tag=f"wbb{l}")
                nc.vector.tensor_copy(out=wb[:, :Dn], in_=wb_ps[0:1, :Dn])
                wbb[l + 1] = wb

            for _rep in range(repeat):
                try:
                    ck("setup")
                    # ======== layer 1 ========
                    ssum1 = stpool.tile([F1, nb], f32, tag="ssum1")
                    ssq1 = stpool.tile([F1, nb], f32, tag="ssq1")
                    message_pass(1, post_factory(1, ssum1, ssq1))
                    ck("mp1")
                    bn_phase(1, F1, ssum1, ssq1)
                    ck("bn1")

                    # ======== layer 2 ========
                    ssum2 = stpool.tile([F2, nb], f32, tag="ssum2")
                    ssq2 = stpool.tile([F2, nb], f32, tag="ssq2")
                    message_pass(2, post_factory(2, ssum2, ssq2))
                    ck("mp2")
                    bn_phase(2, F2, ssum2, ssq2)
                    ck("bn2")

                    # ======== layer 3 ========
                    ssum3 = stpool.tile([F3, nb], f32, tag="ssum3")
                    ssq3 = stpool.tile([F3, nb], f32, tag="ssq3")
                    message_pass(3, post_factory(3, ssum3, ssq3))
                    ck("mp3")
                    bn_phase(3, F3, ssum3, ssq3)
                    ck("bn3")

                    # ======== layer 4 ========
                    logits = stpool.tile([P, nb, 2], f32, tag="logits")
                    message_pass(4, post_factory(4, None, None, logits))
                    ck("mp4")

                    lb = stpool.tile([P, nb, 2], f32, tag="lb")
                    nc.vector.tensor_tensor(
                        out=lb[:], in0=logits[:],
                        in1=b4r[:].to_broadcast([P, nb, 2]), op=Alu.add)
                    mx = stpool.tile([P, nb, 1], f32, tag="mx")
                    nc.vector.tensor_reduce(out=mx[:], in_=lb[:], axis=Axis.X,
                                            op=Alu.max)
                    dz = stpool.tile([P, nb, 2], f32, tag="dz")
                    nc.vector.tensor_tensor(out=dz[:], in0=lb[:],
                                            in1=mx[:].to_broadcast([P, nb, 2]),
                                            op=Alu.subtract)
                    ez = stpool.tile([P, nb, 2], f32, tag="ez")
                    nc.scalar.activation(out=ez[:], in_=dz[:], func=Act.Exp)
                    se = stpool.tile([P, nb, 1], f32, tag="se")
                    nc.vector.tensor_reduce(out=se[:], in_=ez[:], axis=Axis.X,
                                            op=Alu.add)
                    ls = stpool.tile([P, nb, 1], f32, tag="ls")
                    nc.scalar.activation(out=ls[:], in_=se[:], func=Act.Ln)
                    ov = stpool.tile([P, nb, 2], f32, tag="ov")
                    nc.vector.tensor_tensor(out=ov[:], in0=dz[:],
                                            in1=ls[:].to_broadcast([P, nb, 2]),
                                            op=Alu.subtract)
                    nc.sync.dma_start(out=out_p[:], in_=ov[:])
                except _Stop:
                    zz = stpool.tile([P, nb, 2], f32, tag="zz")
                    nc.vector.memset(zz[:], 0.0)
                    nc.sync.dma_start(out=out_p[:], in_=zz[:])

    nc.compile()
    return nc


# ------------------------------------------------------------------ driver

_CACHE = {}


def _loc2glob(plan, c):
    """Global gpos for core c's local rows 0..npc-1 (block-major)."""
    npc = plan["npc"]
    return c * npc + np.arange(npc)


def _prep_in_maps(plan, inputs):
    n_cores, npc, nb = plan["n_cores"], plan["npc"], plan["nb"]
    N = plan["N"]
    dinv = plan["dinv"]
    stld = plan["stld"]

    x = np.asarray(inputs["x"], np.float32)
    xpad = np.zeros((plan["npad"], 64), np.float32)
    xpad[plan["newid"][:N], :x.shape[1]] = x

    W1 = np.asarray(inputs["W1"], np.float32)
    W1p = np.zeros((64, 128), np.float16)
    W1p[:W1.shape[0]] = W1.astype(np.float16)
    coliota = np.broadcast_to(np.arange(P, dtype=np.float16), (P, P)).copy()

    def col(v):
        return np.ascontiguousarray(np.asarray(v, np.float32).reshape(-1, 1))

    common = {
        "W1p": W1p, "W2": np.asarray(inputs["W2"], np.float32),
        "W3": np.asarray(inputs["W3"], np.float32),
        "W4": np.asarray(inputs["W4"], np.float32),
        "coliota": coliota,
        "ident16": np.eye(P, dtype=np.float16),
        "ident": np.eye(P, dtype=np.float32),
        "b4r": np.broadcast_to(np.asarray(inputs["b4"], np.float32),
                               (P, 2)).copy(),
    }
    K = plan["npad"] - N
    for l in (1, 2, 3):
        b = np.asarray(inputs[f"b{l}"], np.float32)
        common[f"b{l}"] = col(b)
        common[f"g{l}"] = col(inputs[f"g{l}"])
        common[f"be{l}"] = col(inputs[f"be{l}"])
        rb = np.maximum(b, 0.0)
        common[f"cs{l}"] = col(-K * rb / n_cores)
        common[f"cq{l}"] = col(-K * rb * rb / n_cores)

    in_maps = []
    for c in range(n_cores):
        g = _loc2glob(plan, c)
        dc = dinv[g]
        m = dict(common)
        m["x_own"] = np.ascontiguousarray(xpad[g])
        m["idx"] = plan["idx_arrs"][c]
        m["slot"] = plan["slot_arrs"][c].astype(np.float16)
        m["dinv_rep"] = np.ascontiguousarray(np.broadcast_to(dc, (P, npc)))
        m["dinv_nm"] = np.ascontiguousarray(dc.reshape(nb, P).T)
        m["srow"] = np.ascontiguousarray(
            stld[g].astype(np.float16).reshape(1, npc))
        in_maps.append(m)
    return in_maps


def _unshard(plan, outs):
    nb, npc, N = plan["nb"], plan["npc"], plan["N"]
    full = np.zeros((plan["npad"], 2), np.float32)
    for c in range(plan["n_cores"]):
        o = np.asarray(outs[c]["out"]).reshape(P, nb, 2)
        full[_loc2glob(plan, c)] = \
            np.ascontiguousarray(o.transpose(1, 0, 2)).reshape(npc, 2)
    return full[plan["newid"][:N]]


LAST_EXEC_NS = None


def kernel(**inputs):
    global LAST_EXEC_NS
    from concourse.bass_utils import run_bass_kernel_spmd

    edge_index = np.asarray(inputs["edge_index"])
    N = int(np.asarray(inputs["x"]).shape[0])
    plan = _build_plan(edge_index, N, 8)

    key = ("v4", N, plan["tott"],
           tuple((b["t0"], b["nt"]) for b in plan["batches"]),
           tuple(c for b in plan["batches"] for c in b["calls"]))
    if key not in _CACHE:
        _CACHE[key] = _build_nc(plan)
    nc = _CACHE[key]

    in_maps = _prep_in_maps(plan, inputs)
    trace = bool(int(os.environ.get("GCN_TRACE", "0")))
    res = run_bass_kernel_spmd(nc, in_maps, list(range(8)), trace=trace)
    LAST_EXEC_NS = res.exec_time_ns
    return _unshard(plan, res.results)


# revision 12
# speedup vs baseline: 1.1539x; 1.1539x over previous
"""DisenGCN-style 4-layer GCN on 8 Trainium2 NeuronCores.

Algorithm (matches reference.py):
    src,dst,norm = gcn_norm(edge_index)  with self loops, norm=dinv[src]*dinv[dst]
    h = BN(relu(conv(x, W1)));  h = BN(relu(conv(h, W2)))
    h = BN(relu(conv(h, W3)));  out = log_softmax(conv(h, W4))

Key algebraic restructuring:
  * norm factorizes: fold dinv into a "message table" T = dinv * H (per-node
    row scale).  Then  agg[d] = dinv[d] * sum_{e: dst=d} T[src_e]  with self
    loops appended as ordinary edges.  No per-edge multiply remains.
  * conv1 / conv4 propagate BEFORE their GEMM (linearity): propagate x
    (24 -> pad 64 dims) and h3 (64 dims) instead of 128 / 2 dims.

Distribution (8 cores, SPMD single program):
  * nodes block-partitioned: core c owns rows [c*NPC, (c+1)*NPC).
  * each core handles edges whose dst it owns; message tables are replicated
    via AllGather after each layer; BN stats via a tiny AllReduce.
  * per-edge gather uses dma_gather (int16 indices -> table is split into 4
    source buckets < 32768 rows; edges grouped by (batch of dst blocks,
    bucket, dst block), each (block,bucket) group padded to 128-edge tiles).
  * scatter-add is a one-hot matmul: per 128-edge tile, S[e, slot] =
    (dst_slot[e] == slot) built on DVE; PSUM accumulates
    acc[f, slot] += sum_e M[e, f] * S[e, slot] over the block's tiles.
"""

import os
import sys
import math
import numpy as np

sys.path.insert(0, "/opt/trn_rl_repo")

P = 128
GB = 6            # dst blocks per gather batch
BUCKET_MAX = 32768  # dma_gather int16 index limit (overridable for tests)
MP_MODE = "full"    # debug: gather | smat | mm | full
F16 = bool(int(os.environ.get("GCN_F16", "1")))  # fp16 message tables


# ---------------------------------------------------------------- host prep


def _build_plan(edge_index, N, n_cores):
    """Partition edges; build per-core index/slot streams + shared structure."""
    npc = int(math.ceil(N / n_cores / P)) * P          # nodes per core
    npad = npc * n_cores
    nb = npc // P                                       # dst blocks per core
    nbuk = max(1, int(math.ceil(npad / BUCKET_MAX)))    # source buckets
    while npad % nbuk:
        nbuk += 1
    bs = npad // nbuk
    assert bs <= BUCKET_MAX

    src0 = np.asarray(edge_index[0], dtype=np.int64)
    dst0 = np.asarray(edge_index[1], dtype=np.int64)
    # self loops are handled by a direct DMA + identity matmul (no gather),
    # so the edge streams carry only the directed edges.
    src = src0
    dst = dst0

    # relabel nodes so each (core, block) bin carries a near-equal edge load:
    # greedy heaviest-first assignment to the lightest non-full bin.
    import heapq
    degN = np.bincount(dst, minlength=N)
    nbins = n_cores * nb
    order = np.argsort(-degN, kind="stable")
    heap = [(0, b) for b in range(nbins)]
    heapq.heapify(heap)
    cap = np.full(nbins, P, np.int64)
    newid = np.empty(npad, np.int64)
    base = (np.arange(nbins) // nb) * npc + (np.arange(nbins) % nb) * P
    for n in order:
        while True:
            load, b = heapq.heappop(heap)
            if cap[b] > 0:
                break
        newid[n] = base[b] + (P - cap[b])
        cap[b] -= 1
        heapq.heappush(heap, (load + int(degN[n]), b))
    spots = np.concatenate([np.arange(base[b] + P - cap[b], base[b] + P)
                            for b in range(nbins)]) if cap.sum() else \
        np.empty(0, np.int64)
    newid[N:] = spots
    src = newid[src]
    dst = newid[dst]

    # degree includes the self loop
    deg = np.bincount(dst, minlength=npad).astype(np.float64)
    deg[newid[:N]] += 1.0
    dinv = np.zeros(npad, np.float32)
    nz = deg > 0
    dinv[nz] = (1.0 / np.sqrt(deg[nz])).astype(np.float32)

    ngrp = nb * nbuk
    core_data = []
    counts = np.zeros((n_cores, ngrp), np.int64)
    for c in range(n_cores):
        m = (dst >= c * npc) & (dst < (c + 1) * npc)
        s = src[m]
        d = dst[m] - c * npc
        key = (d >> 7) * nbuk + s // bs
        order = np.argsort(key, kind="stable")
        counts[c] = np.bincount(key, minlength=ngrp)
        core_data.append((key[order], (s % bs)[order], (d & 127)[order]))

    tiles_grp = (counts.max(axis=0) + P - 1) // P       # tiles per (block,bucket)

    # stream order: for each batch of GB blocks: for bucket: for block
    n_batches = (nb + GB - 1) // GB
    grp_order = []
    batches = []    # per batch: dict(t0, nt, blks, calls=[(k,t0,nt)], bt=..)
    tpos = 0
    for g in range(n_batches):
        blks = list(range(g * GB, min((g + 1) * GB, nb)))
        b_t0 = tpos
        bcalls = []
        btiles = {b: [] for b in blks}     # (bucket, t0, nt) per block
        for k in range(nbuk):
            c_t0 = tpos
            for b in blks:
                t = int(tiles_grp[b * nbuk + k])
                if t:
                    grp_order.append(b * nbuk + k)
                    btiles[b].append((k, tpos, t))
                    tpos += t
            # dma_gather tops out at 1024 indices (= 8 tiles) per call
            for sub in range(c_t0, tpos, 8):
                bcalls.append((k, sub, min(8, tpos - sub)))
        batches.append(dict(t0=b_t0, nt=tpos - b_t0, blks=blks,
                            calls=bcalls, btiles=btiles))
    tott = tpos
    tote = tott * P

    grp_start = np.full(ngrp, -1, np.int64)
    pos = 0
    for gid in grp_order:
        grp_start[gid] = pos
        pos += int(tiles_grp[gid]) * P

    idx_arrs, slot_arrs = [], []
    for c in range(n_cores):
        key_s, sloc_s, slot_s = core_data[c]
        ne = len(key_s)
        grp_first = np.searchsorted(key_s, np.arange(ngrp), side="left")
        within = np.arange(ne, dtype=np.int64) - grp_first[key_s]
        posi = grp_start[key_s] + within
        idx_stream = np.zeros(tote, np.int16)
        slot_stream = np.full(tote, -1.0, np.float32)
        idx_stream[posi] = sloc_s.astype(np.int16)
        slot_stream[posi] = slot_s.astype(np.float32)
        idx_arrs.append(np.ascontiguousarray(
            np.tile(idx_stream.reshape(-1, 16).T, (8, 1))))
        slot_arrs.append(np.ascontiguousarray(slot_stream.reshape(-1, P).T))

    return dict(
        N=N, n_cores=n_cores, npc=npc, npad=npad, nb=nb, nbuk=nbuk, bs=bs,
        batches=batches, tott=tott, dinv=dinv, newid=newid,
        idx_arrs=idx_arrs, slot_arrs=slot_arrs,
    )


# ------------------------------------------------------------ bass program


class _Stop(Exception):
    pass


def _build_nc(plan, stop_after=None, repeat=1):
    from concourse import bass, mybir, tile, bacc
    f32 = mybir.dt.float32
    f16 = mybir.dt.float16
    dtm = f16 if F16 else f32
    i16 = mybir.dt.int16
    Alu = mybir.AluOpType
    Act = mybir.ActivationFunctionType
    Axis = mybir.AxisListType

    npc, npad, nb, nbuk, bs = (plan[k] for k in ("npc", "npad", "nb", "nbuk", "bs"))
    tott = plan["tott"]
    n_cores = plan["n_cores"]
    N = plan["N"]
    rg = [list(range(n_cores))]
    batches = plan["batches"]
    max_bt = max(b["nt"] for b in batches)                       # tiles per batch
    max_ct = max((nt for b in batches for (_k, _t, nt) in b["calls"]), default=1)

    D1, D2, D3, D4 = 64, 128, 64, 64      # real message content dims
    # gather width: 128 elems in fp16 mode (256B rows; for 64-wide tables the
    # second half of each gathered row is the next node's data -> lands in
    # unused psum rows).  Tables stay content-width + 1 guard row.
    W_ = (lambda d: 128) if F16 else (lambda d: d)
    F1, F2, F3 = 128, 128, 64             # post-conv dims (BN layers)

    nc = bacc.Bacc("TRN2", target_bir_lowering=False, debug=False,
                   num_devices=n_cores, num_swdge_queues=4,
                   dynamic_dma_scratch_size=49152)

    # ---- I/O ----
    totc = tott * 8
    x_own = nc.declare_dram_parameter("x_own", [npc, 64], f32, isOutput=False)
    idx_p = nc.declare_dram_parameter("idx", [P, totc], i16, isOutput=False)
    slot_p = nc.declare_dram_parameter("slot", [P, tott], dtm, isOutput=False)
    dinv_rep_p = nc.declare_dram_parameter("dinv_rep", [P, npc], f32, isOutput=False)
    dinv_nm_p = nc.declare_dram_parameter("dinv_nm", [P, nb], f32, isOutput=False)
    coliota_p = nc.declare_dram_parameter("coliota", [P, P], dtm, isOutput=False)
    ident_p = nc.declare_dram_parameter("ident", [P, P], f32, isOutput=False)
    ident16_p = nc.declare_dram_parameter("ident16", [P, P], dtm, isOutput=False)
    W1_p = nc.declare_dram_parameter("W1p", [64, 128], f32, isOutput=False)
    W2_p = nc.declare_dram_parameter("W2", [128, 128], f32, isOutput=False)
    W3_p = nc.declare_dram_parameter("W3", [128, 64], f32, isOutput=False)
    W4_p = nc.declare_dram_parameter("W4", [64, 2], f32, isOutput=False)
    vecs = {}
    for nm, f in (("b1", F1), ("g1", F1), ("be1", F1), ("cs1", F1), ("cq1", F1),
                  ("b2", F2), ("g2", F2), ("be2", F2), ("cs2", F2), ("cq2", F2),
                  ("b3", F3), ("g3", F3), ("be3", F3), ("cs3", F3), ("cq3", F3)):
        vecs[nm] = nc.declare_dram_parameter(nm, [f, 1], f32, isOutput=False)
    b4r_p = nc.declare_dram_parameter("b4r", [P, 2], f32, isOutput=False)
    out_p = nc.declare_dram_parameter("out", [P, nb, 2], f32, isOutput=True)

    # ---- internal DRAM ----
    T = [None,
         nc.dram_tensor("T1", [npad, W_(D1)], dtm, addr_space="Shared"),
         nc.dram_tensor("T2", [npad, W_(D2)], dtm, addr_space="Shared"),
         nc.dram_tensor("T3", [npad, W_(D3)], dtm, addr_space="Shared"),
         nc.dram_tensor("T4", [npad, W_(D4)], dtm, addr_space="Shared")]
    Tc = [None,
          nc.dram_tensor("T1c", [npc, W_(D1)], dtm),
          nc.dram_tensor("T2c", [npc, W_(D2)], dtm),
          nc.dram_tensor("T3c", [npc, W_(D3)], dtm),
          nc.dram_tensor("T4c", [npc, W_(D4)], dtm)]
    rd = nc.dram_tensor("rd", [nb, P, P], f32)          # relu outs [blk, F, node]
    st_in = [None] + [nc.dram_tensor(f"stin{l}", [[F1, F2, F3][l - 1], 2], f32)
                      for l in (1, 2, 3)]
    st_out = [None] + [nc.dram_tensor(f"stout{l}", [[F1, F2, F3][l - 1], 2], f32,
                                      addr_space="Shared") for l in (1, 2, 3)]

    with tile.TileContext(nc) as tc:
        with (
            tc.tile_pool(name="const", bufs=1) as cpool,
            tc.tile_pool(name="slotp", bufs=1) as slpool,
            tc.tile_pool(name="dinvrep", bufs=1) as dpool,
            tc.tile_pool(name="idxs", bufs=3) as ipool,
            tc.tile_pool(name="msg", bufs=14) as mpool,
            tc.tile_pool(name="smat", bufs=4) as spool,
            tc.tile_pool(name="blk", bufs=2) as bpool,
            tc.tile_pool(name="stats", bufs=1) as stpool,
            tc.tile_pool(name="phb", bufs=2) as hpool,
        ):
            # ---------- constants ----------
            def load_const(handle, shape):
                t = cpool.tile(shape, f32, tag=handle.name)
                nc.sync.dma_start(out=t[:], in_=handle[:])
                return t

            coliota = cpool.tile([P, 1, P], dtm, tag="coliota")
            nc.sync.dma_start(out=coliota[:], in_=coliota_p[:, None, :])
            ident = load_const(ident_p, [P, P])
            ident16 = cpool.tile([P, P], dtm, tag="ident16")
            nc.sync.dma_start(out=ident16[:], in_=ident16_p[:])
            W1s = load_const(W1_p, [64, 128])
            W2s = load_const(W2_p, [128, 128])
            W3s = load_const(W3_p, [128, 64])
            W4s = load_const(W4_p, [64, 2])
            b4r = cpool.tile([P, 1, 2], f32, tag="b4r")
            nc.sync.dma_start(out=b4r[:], in_=b4r_p[:, None, :])
            vt = {nm: load_const(h, list(h.shape)) for nm, h in vecs.items()}
            dinv_nm = load_const(dinv_nm_p, [P, nb])

            slot_sb = slpool.tile([P, tott, 1], dtm)
            nc.sync.dma_start(out=slot_sb[:], in_=slot_p[:, :, None])
            dinv_rep = dpool.tile([P, npc], f32)
            nc.sync.dma_start(out=dinv_rep[:], in_=dinv_rep_p[:])

            def ck(name):
                if stop_after == name:
                    raise _Stop()

            # fp16 mode: tables are 128 wide; zero the unused pad columns once
            if F16:
                zpad = cpool.tile([P, 64], dtm, tag="zpad")
                nc.vector.memset(zpad[:], 0.0)
                for li, d in ((1, D1), (3, D3), (4, D4)):
                    for b in range(nb):
                        nc.sync.dma_start(
                            out=Tc[li][b * P:(b + 1) * P, d:],
                            in_=zpad[:, :128 - d])

            # ---------- T1 = dinv * x_pad (own slice) -> AllGather ----------
            for b in range(nb):
                xb = bpool.tile([P, D1], f32, tag="r0")
                nc.sync.dma_start(out=xb[:], in_=x_own[b * P:(b + 1) * P, :])
                t1b = bpool.tile([P, D1], dtm, tag="t1b")
                nc.vector.tensor_tensor(
                    out=t1b[:], in0=xb[:],
                    in1=dinv_nm[:, b:b + 1].to_broadcast([P, D1]),
                    op=Alu.mult)
                nc.sync.dma_start(out=Tc[1][b * P:(b + 1) * P, :D1], in_=t1b[:])
            nc.gpsimd.collective_compute(
                "AllGather", Alu.bypass, replica_groups=rg,
                ins=[Tc[1].ap().opt()], outs=[T[1].ap().opt()])

            # ---------- helpers ----------
            gq = [0]   # global gather counter: Tile locks DMASW lane (mod 8)
                       # to SWDGE queue, so queue must follow the same counter

            def message_pass(D, table, post_block, ps_acc, tagsuf, tcn):
                for bi, binfo in enumerate(batches):
                    bt0, bnt, blks = binfo["t0"], binfo["nt"], binfo["blks"]
                    if bnt == 0:
                        continue
                    idxb = ipool.tile([P, max_bt * 8], i16, tag="idxb")
                    nc.sync.dma_start(out=idxb[:, :bnt * 8],
                                      in_=idx_p[:, bt0 * 8:(bt0 + bnt) * 8])
                    DW = W_(D)
                    msgs = {}
                    for ci, (k, ct0, cnt) in enumerate(binfo["calls"]):
                        nidx = cnt * P
                        msgs[ci] = mpool.tile([P, 8, DW], dtm, tag="msg",
                                              name=f"msg{ci}")
                        nc.gpsimd.dma_gather(
                            out_ap=msgs[ci][:, :cnt, :],
                            in_ap=table[k * bs:(k + 1) * bs, :],
                            idxs_ap=idxb[:, (ct0 - bt0) * 8:(ct0 - bt0 + cnt) * 8],
                            num_idxs=nidx, num_idxs_reg=nidx, elem_size=DW,
                            queue_num=gq[0] % 4)
                        gq[0] += 1
                    if MP_MODE == "gather":
                        continue
                    accs = {}
                    first = {b: False for b in blks}
                    last_mm = {b: None for b in blks}
                    for b in blks:
                        accs[b] = ps_acc.tile([DW, P], f32, tag="acc" + tagsuf,
                                              name=f"acc{b}")
                        last_mm[b] = binfo["btiles"][b][-1][1] + \
                            binfo["btiles"][b][-1][2] - 1 if binfo["btiles"][b] else None
                        # self-loop term: plain DMA of the block's own table
                        # rows + identity matmul (acc[:, j] += T[row j])
                        ms = mpool.tile([P, W_(D)], dtm, tag="mself",
                                        name=f"mself{b}")
                        nc.sync.dma_start(out=ms[:],
                                          in_=tcn[b * P:(b + 1) * P, :])
                        nc.tensor.matmul(out=accs[b][:], lhsT=ms[:],
                                         rhs=ident16[:],
                                         start=True, stop=(last_mm[b] is None))
                    for ci, (k, ct0, cnt) in enumerate(binfo["calls"]):
                        S = spool.tile([P, 8, P], dtm, tag="S", name=f"S{ci}")
                        nc.vector.tensor_tensor(
                            out=S[:, :cnt, :],
                            in0=slot_sb[:, ct0:ct0 + cnt, :].to_broadcast([P, cnt, P]),
                            in1=coliota[:].to_broadcast([P, cnt, P]),
                            op=Alu.is_equal)
                        if MP_MODE == "smat":
                            continue
                        for b in blks:
                            for (kk, t0, nt) in binfo["btiles"][b]:
                                if kk != k:
                                    continue
                                lo = max(t0, ct0)
                                hi = min(t0 + nt, ct0 + cnt)
                                for t in range(lo, hi):
                                    nc.tensor.matmul(
                                        out=accs[b][:],
                                        lhsT=msgs[ci][:, t - ct0, :],
                                        rhs=S[:, t - ct0, :],
                                        start=first[b], stop=(t == last_mm[b]))
                                    first[b] = False
                    if MP_MODE in ("smat", "mm"):
                        continue
                    for b in blks:
                        post_block(b, accs[b])

            def bn_phase(layer, F, ssum, ssq):
                stat = stpool.tile([F, 2], f32, tag=f"stat{layer}")
                nc.vector.tensor_reduce(out=stat[:, 0:1], in_=ssum[:],
                                        axis=Axis.X, op=Alu.add)
                nc.vector.tensor_reduce(out=stat[:, 1:2], in_=ssq[:],
                                        axis=Axis.X, op=Alu.add)
                nc.vector.tensor_tensor(out=stat[:, 0:1], in0=stat[:, 0:1],
                                        in1=vt[f"cs{layer}"][:], op=Alu.add)
                nc.vector.tensor_tensor(out=stat[:, 1:2], in0=stat[:, 1:2],
                                        in1=vt[f"cq{layer}"][:], op=Alu.add)
                nc.sync.dma_start(out=st_in[layer][:], in_=stat[:])
                nc.gpsimd.collective_compute(
                    "AllReduce", Alu.add, replica_groups=rg,
                    ins=[st_in[layer].ap().opt()], outs=[st_out[layer].ap().opt()])
                stg = stpool.tile([F, 2], f32, tag=f"statg{layer}")
                nc.sync.dma_start(out=stg[:], in_=st_out[layer][:])
                mean = stpool.tile([F, 1], f32, tag=f"mean{layer}")
                nc.vector.tensor_scalar(out=mean[:], in0=stg[:, 0:1],
                                        scalar1=1.0 / N, scalar2=None, op0=Alu.mult)
                var = stpool.tile([F, 1], f32, tag=f"var{layer}")
                nc.vector.tensor_scalar(out=var[:], in0=stg[:, 1:2],
                                        scalar1=1.0 / N, scalar2=None, op0=Alu.mult)
                msq = stpool.tile([F, 1], f32, tag=f"msq{layer}")
                nc.vector.tensor_tensor(out=msq[:], in0=mean[:], in1=mean[:],
                                        op=Alu.mult)
                nc.vector.tensor_tensor(out=var[:], in0=var[:], in1=msq[:],
                                        op=Alu.subtract)
                ve = stpool.tile([F, 1], f32, tag=f"ve{layer}")
                nc.vector.tensor_scalar(out=ve[:], in0=var[:], scalar1=1e-5,
                                        scalar2=None, op0=Alu.add)
                sd = stpool.tile([F, 1], f32, tag=f"sd{layer}")
                nc.scalar.activation(out=sd[:], in_=ve[:], func=Act.Sqrt)
                inv = stpool.tile([F, 1], f32, tag=f"inv{layer}")
                nc.vector.reciprocal(out=inv[:], in_=sd[:])
                a = stpool.tile([F, 1], f32, tag=f"a{layer}")
                nc.vector.tensor_tensor(out=a[:], in0=vt[f"g{layer}"][:],
                                        in1=inv[:], op=Alu.mult)
                am = stpool.tile([F, 1], f32, tag=f"am{layer}")
                nc.vector.tensor_tensor(out=am[:], in0=a[:], in1=mean[:],
                                        op=Alu.mult)
                bb = stpool.tile([F, 1], f32, tag=f"bb{layer}")
                nc.vector.tensor_tensor(out=bb[:], in0=vt[f"be{layer}"][:],
                                        in1=am[:], op=Alu.subtract)
                return a, bb

            def table_build(F, a, bb, Wn, Dn, tcn, tn, ps_gem, ps_tr):
                for ch0 in range(0, nb, 2):
                    nbl = min(2, nb - ch0)
                    w = nbl * P
                    h = hpool.tile([F, 256], f32, tag="h")
                    for j in range(nbl):
                        b = ch0 + j
                        nc.sync.dma_start(out=h[:, j * P:(j + 1) * P],
                                          in_=rd[b, :F, :])
                    ha = hpool.tile([F, 256], f32, tag="ha")
                    nc.scalar.activation(out=ha[:, :w], in_=h[:, :w],
                                         func=Act.Identity,
                                         bias=bb[:], scale=a[:])
                    if Wn is not None:
                        gp = ps_gem.tile([Dn, 256], f32, tag="gemB")
                        nc.tensor.matmul(out=gp[:, :w], lhsT=Wn[:], rhs=ha[:, :w],
                                         start=True, stop=True)
                        gs = hpool.tile([Dn, 256], f32, tag="gs")
                        nc.vector.tensor_copy(out=gs[:, :w], in_=gp[:, :w])
                    else:
                        gs = ha
                    for j in range(nbl):
                        b = ch0 + j
                        pt = ps_tr.tile([P, Dn], f32, tag="ptr")
                        nc.tensor.transpose(out=pt[:], in_=gs[:Dn, j * P:(j + 1) * P],
                                            identity=ident[:Dn, :Dn])
                        tt = hpool.tile([P, Dn], dtm, tag="tt")
                        nc.vector.tensor_tensor(
                            out=tt[:], in0=pt[:],
                            in1=dinv_nm[:, b:b + 1].to_broadcast([P, Dn]),
                            op=Alu.mult)
                        nc.sync.dma_start(out=tcn[b * P:(b + 1) * P, :Dn], in_=tt[:])
                nc.gpsimd.collective_compute(
                    "AllGather", Alu.bypass, replica_groups=rg,
                    ins=[tcn.ap().opt()], outs=[tn.ap().opt()])

            for _rep in range(repeat):
                try:
                    # ======== layer 1 ========
                    ck("setup")
                    ssum1 = stpool.tile([F1, nb], f32, tag="ssum1")
                    ssq1 = stpool.tile([F1, nb], f32, tag="ssq1")
                    with tc.tile_pool(name="ps1", bufs=6, space="PSUM") as ps_acc, \
                         tc.tile_pool(name="ps1g", bufs=2, space="PSUM") as ps_gem:

                        def post1(b, acc):
                            s1 = bpool.tile([64, P], f32, tag="s1")
                            nc.vector.tensor_tensor(
                                out=s1[:], in0=acc[:64, :],
                                in1=dinv_rep[:64, b * P:(b + 1) * P], op=Alu.mult)
                            z = ps_gem.tile([128, P], f32, tag="z1")
                            nc.tensor.matmul(out=z[:], lhsT=W1s[:], rhs=s1[:],
                                             start=True, stop=True)
                            r = bpool.tile([F1, P], f32, tag="r")
                            nc.scalar.activation(out=r[:], in_=z[:], func=Act.Relu,
                                                 bias=vt["b1"][:], scale=1.0,
                                                 accum_out=ssum1[:, b:b + 1])
                            sq = bpool.tile([F1, P], f32, tag="sq")
                            nc.scalar.activation(out=sq[:], in_=r[:], func=Act.Square,
                                                 accum_out=ssq1[:, b:b + 1])
                            nc.sync.dma_start(out=rd[b, :F1, :], in_=r[:])

                        message_pass(D1, T[1], post1, ps_acc, "a", Tc[1])
                    ck("mp1")
                    a1, bb1 = bn_phase(1, F1, ssum1, ssq1)
                    ck("bn1")
                    with tc.tile_pool(name="pb1g", bufs=2, space="PSUM") as ps_gem, \
                         tc.tile_pool(name="pb1t", bufs=2, space="PSUM") as ps_tr:
                        table_build(F1, a1, bb1, W2s, D2, Tc[2], T[2], ps_gem, ps_tr)
                    ck("tb1")

                    # ======== layer 2 ========
                    ssum2 = stpool.tile([F2, nb], f32, tag="ssum2")
                    ssq2 = stpool.tile([F2, nb], f32, tag="ssq2")
                    with tc.tile_pool(name="ps2", bufs=6, space="PSUM") as ps_acc:

                        def post2(b, acc):
                            t2 = bpool.tile([F2, P], f32, tag="r0")
                            nc.vector.tensor_tensor(
                                out=t2[:], in0=acc[:F2, :],
                                in1=dinv_rep[:F2, b * P:(b + 1) * P], op=Alu.mult)
                            r = bpool.tile([F2, P], f32, tag="r")
                            nc.scalar.activation(out=r[:], in_=t2[:], func=Act.Relu,
                                                 bias=vt["b2"][:], scale=1.0,
                                                 accum_out=ssum2[:, b:b + 1])
                            sq = bpool.tile([F2, P], f32, tag="sq")
                            nc.scalar.activation(out=sq[:], in_=r[:], func=Act.Square,
                                                 accum_out=ssq2[:, b:b + 1])
                            nc.sync.dma_start(out=rd[b, :F2, :], in_=r[:])

                        message_pass(D2, T[2], post2, ps_acc, "a", Tc[2])
                    ck("mp2")
                    a2, bb2 = bn_phase(2, F2, ssum2, ssq2)
                    ck("bn2")
                    with tc.tile_pool(name="pb2g", bufs=2, space="PSUM") as ps_gem, \
                         tc.tile_pool(name="pb2t", bufs=2, space="PSUM") as ps_tr:
                        table_build(F2, a2, bb2, W3s, D3, Tc[3], T[3], ps_gem, ps_tr)
                    ck("tb2")

                    # ======== layer 3 ========
                    ssum3 = stpool.tile([F3, nb], f32, tag="ssum3")
                    ssq3 = stpool.tile([F3, nb], f32, tag="ssq3")
                    with tc.tile_pool(name="ps3", bufs=6, space="PSUM") as ps_acc:

                        def post3(b, acc):
                            t3 = bpool.tile([F3, P], f32, tag="r0")
                            nc.vector.tensor_tensor(
                                out=t3[:], in0=acc[:F3, :],
                                in1=dinv_rep[:F3, b * P:(b + 1) * P], op=Alu.mult)
                            r = bpool.tile([F3, P], f32, tag="r")
                            nc.scalar.activation(out=r[:], in_=t3[:], func=Act.Relu,
                                                 bias=vt["b3"][:], scale=1.0,
                                                 accum_out=ssum3[:, b:b + 1])
                            sq = bpool.tile([F3, P], f32, tag="sq")
                            nc.scalar.activation(out=sq[:], in_=r[:], func=Act.Square,
                                                 accum_out=ssq3[:, b:b + 1])
                            nc.sync.dma_start(out=rd[b, :F3, :], in_=r[:])

                        message_pass(D3, T[3], post3, ps_acc, "a", Tc[3])
                    ck("mp3")
                    a3, bb3 = bn_phase(3, F3, ssum3, ssq3)
                    ck("bn3")
                    with tc.tile_pool(name="pb3g", bufs=2, space="PSUM") as ps_gem, \
                         tc.tile_pool(name="pb3t", bufs=2, space="PSUM") as ps_tr:
                        table_build(F3, a3, bb3, None, D4, Tc[4], T[4], ps_gem, ps_tr)
                    ck("tb3")

                    # ======== layer 4 ========
                    logits = stpool.tile([P, nb, 2], f32, tag="logits")
                    with tc.tile_pool(name="ps4", bufs=6, space="PSUM") as ps_acc, \
                         tc.tile_pool(name="ps4g", bufs=2, space="PSUM") as ps_gem:

                        def post4(b, acc):
                            s4 = bpool.tile([64, P], f32, tag="s1")
                            nc.vector.tensor_tensor(
                                out=s4[:], in0=acc[:64, :],
                                in1=dinv_rep[:64, b * P:(b + 1) * P], op=Alu.mult)
                            lg = ps_gem.tile([P, 2], f32, tag="lg")
                            nc.tensor.matmul(out=lg[:], lhsT=s4[:], rhs=W4s[:],
                                             start=True, stop=True)
                            nc.vector.tensor_copy(out=logits[:, b, :], in_=lg[:])

                        message_pass(D4, T[4], post4, ps_acc, "a", Tc[4])
                    ck("mp4")

                    lb = stpool.tile([P, nb, 2], f32, tag="lb")
                    nc.vector.tensor_tensor(
                        out=lb[:], in0=logits[:],
                        in1=b4r[:].to_broadcast([P, nb, 2]), op=Alu.add)
                    mx = stpool.tile([P, nb, 1], f32, tag="mx")
                    nc.vector.tensor_reduce(out=mx[:], in_=lb[:], axis=Axis.X, op=Alu.max)
                    dz = stpool.tile([P, nb, 2], f32, tag="dz")
                    nc.vector.tensor_tensor(out=dz[:], in0=lb[:],
                                            in1=mx[:].to_broadcast([P, nb, 2]),
                                            op=Alu.subtract)
                    ez = stpool.tile([P, nb, 2], f32, tag="ez")
                    nc.scalar.activation(out=ez[:], in_=dz[:], func=Act.Exp)
                    se = stpool.tile([P, nb, 1], f32, tag="se")
                    nc.vector.tensor_reduce(out=se[:], in_=ez[:], axis=Axis.X, op=Alu.add)
                    ls = stpool.tile([P, nb, 1], f32, tag="ls")
                    nc.scalar.activation(out=ls[:], in_=se[:], func=Act.Ln)
                    ov = stpool.tile([P, nb, 2], f32, tag="ov")
                    nc.vector.tensor_tensor(out=ov[:], in0=dz[:],
                                            in1=ls[:].to_broadcast([P, nb, 2]),
                                            op=Alu.subtract)
                    nc.sync.dma_start(out=out_p[:], in_=ov[:])
                except _Stop:
                    zz = stpool.tile([P, nb, 2], f32, tag="zz")
                    nc.vector.memset(zz[:], 0.0)
                    nc.sync.dma_start(out=out_p[:], in_=zz[:])



    nc.compile()
    return nc


# ------------------------------------------------------------------ driver

_CACHE = {}


def _prep_in_maps(plan, inputs):
    n_cores, npc, nb = plan["n_cores"], plan["npc"], plan["nb"]
    N = plan["N"]
    dinv = plan["dinv"]

    x = np.asarray(inputs["x"], np.float32)
    xpad = np.zeros((plan["npad"], 64), np.float32)
    xpad[plan["newid"][:N], :x.shape[1]] = x

    W1 = np.asarray(inputs["W1"], np.float32)
    W1p = np.zeros((64, 128), np.float32)
    W1p[:W1.shape[0]] = W1
    dt_m = np.float16 if F16 else np.float32
    coliota = np.broadcast_to(np.arange(P, dtype=dt_m), (P, P)).copy()
    ident = np.eye(P, dtype=np.float32)

    def col(v):
        return np.ascontiguousarray(np.asarray(v, np.float32).reshape(-1, 1))

    common = {
        "W1p": W1p, "W2": np.asarray(inputs["W2"], np.float32),
        "W3": np.asarray(inputs["W3"], np.float32),
        "W4": np.asarray(inputs["W4"], np.float32),
        "coliota": coliota, "ident": ident,
        "ident16": np.eye(P, dtype=dt_m),
        "b4r": np.broadcast_to(np.asarray(inputs["b4"], np.float32), (P, 2)).copy(),
    }
    K = plan["npad"] - N
    for l in (1, 2, 3):
        b = np.asarray(inputs[f"b{l}"], np.float32)
        common[f"b{l}"] = col(b)
        common[f"g{l}"] = col(inputs[f"g{l}"])
        common[f"be{l}"] = col(inputs[f"be{l}"])
        rb = np.maximum(b, 0.0)
        common[f"cs{l}"] = col(-K * rb / n_cores)
        common[f"cq{l}"] = col(-K * rb * rb / n_cores)

    in_maps = []
    for c in range(n_cores):
        dc = dinv[c * npc:(c + 1) * npc]
        m = dict(common)
        m["x_own"] = np.ascontiguousarray(xpad[c * npc:(c + 1) * npc])
        m["idx"] = plan["idx_arrs"][c]
        m["slot"] = plan["slot_arrs"][c].astype(dt_m)
        m["dinv_rep"] = np.ascontiguousarray(np.broadcast_to(dc, (P, npc)))
        m["dinv_nm"] = np.ascontiguousarray(dc.reshape(nb, P).T)
        in_maps.append(m)
    return in_maps


def _unshard(plan, outs):
    nb, npc, N = plan["nb"], plan["npc"], plan["N"]
    parts = []
    for c in range(plan["n_cores"]):
        o = np.asarray(outs[c]["out"]).reshape(P, nb, 2)
        parts.append(np.ascontiguousarray(o.transpose(1, 0, 2)).reshape(npc, 2))
    full = np.concatenate(parts, axis=0)
    return full[plan["newid"][:N]]


LAST_EXEC_NS = None


def kernel(**inputs):
    global LAST_EXEC_NS
    from concourse.bass_utils import run_bass_kernel_spmd

    edge_index = np.asarray(inputs["edge_index"])
    N = int(np.asarray(inputs["x"]).shape[0])
    plan = _build_plan(edge_index, N, 8)

    key = ("v2", N, plan["tott"], F16,
           tuple((b["t0"], b["nt"]) for b in plan["batches"]),
           tuple(c for b in plan["batches"] for c in b["calls"]))
    if key not in _CACHE:
        _CACHE[key] = _build_nc(plan)
    nc = _CACHE[key]

    in_maps = _prep_in_maps(plan, inputs)
    trace = bool(int(os.environ.get("GCN_TRACE", "0")))
    res = run_bass_kernel_spmd(nc, in_maps, list(range(8)), trace=trace)
    LAST_EXEC_NS = res.exec_time_ns
    return _unshard(plan, res.results)

